# revision 28
# baseline (speedup 1.0000x reference)
"""Locally-connected autoencoder (128 independent 256->8->256 per-patch linears
+ sigmoid) on 8 Trainium2 NeuronCores.

Strategy
--------
Pure data parallel over the feature bands: core k owns image bands (2k, 2k+1)
for ALL 2048 samples.  The problem is HBM-bound (the 2.15 GFLOP of matmul is
trivial next to 512 MB of fp32 I/O), so the whole game is minimizing DMA
bytes:

* x ships as a SINGLE fp16 tensor (2 B/elem) instead of fp32 or a hi/lo bf16
  pair.  fp16's 10-bit mantissa keeps the max output rel-err ~4e-3, far
  inside the 2e-2 gate.  Host pre-transposes to [band, n-half, w, r, n'] so
  DMA descriptors move 8-32 KiB contiguous on both the HBM and SBUF side.
* the output is written back as fp16 (sigmoid output is in (0,1): always
  fp16-normal since |logit| <~ 6) and upcast on host - halves output bytes.
* the encode bias is folded into the decode bias on host
  (bd' = bd + Wd @ be), so the z path is a pure PSUM->SBUF fp16 cast (DVE)
  and ACT does nothing but Sigmoid+bias.

Per (band, n-half) unit - 2048 features x 1024 samples:
  encode:  z[64(pw,h), n1024] = sum_r Wenc_bd[r].T @ X[r]  (32 fp16 matmuls
           accumulated in PSUM; Wenc_bd is the block-diag arrangement of We)
  cast:    DVE copies z PSUM -> SBUF fp16 (no bias needed).
  decode:  out^T[128 f, n512] = Wdec_bd[:, fchunk].T @ z   (2 fp16 matmuls
           per fchunk; fchunk = one image row r')
  sigmoid: ACT reads decode PSUM [128, 1024], adds bd' as per-partition
           bias, writes fp16 out^T tiles -> DMA to HBM.

Scheduling (the whole kernel is a 4-stage software pipeline over units
(0,0),(0,1),(1,0),(1,1), paced by two facts measured from traces):
* DMA queues are FIFO per ring: input issued up-front blocks every output
  store behind it, stalling the o_sb recycle and ACT.  So only unit (0,0)'s
  x + band-0 weights + x(0,1) issue up-front; x(1,0)/x(1,1) r-quarters and
  band-1 weights issue from inside earlier decode loops, spaced >= 3 stores
  apart so each 1 MB quarter's FIFO block is absorbed by the o_sb depth.
* PE is in-order and ACT-backpressured during decode, so the next unit's
  encode matmuls are emitted interleaved between decode matmuls (start_fcs
  chosen so the thunks never wait on x, which would stall the stream), and
  finish just as ACT drains the current unit - ACT never idles between
  units.  x(0,0) streams as r-quarters so encode overlaps its own DMA.

Engine split: SP issues all DMAs (stores after their ACT sem, giving
just-in-time issue pacing), PE matmuls, DVE z-casts, ACT sigmoids only
(sigmoid table preloaded at t=0; identity/bias work eliminated by the
host-side bias fold).
"""

import numpy as np

# problem constants (hardcoded per contract)
H, W, PS = 256, 128, 16
NPH, NPW = H // PS, W // PS      # 16 bands, 8 patches/band
P, D, HID = NPH * NPW, PS * PS, 8
NSMP = 4 * 512                    # 2048 samples
BANDW = PS * W                    # 2048 floats per band per sample
NCORES = 8
BPC = NPH // NCORES               # 2 bands per core
M = NPW * HID                     # 64 latent rows per band

_PROG = None
LAST_EXEC_NS = None   # filled when kernel() runs with _trace=True


def _install_ntff_hook():
    """The agent image's antenv lacks axon_hooks; synthesize it so
    run_bass_kernel_spmd(trace=True) can capture NTFF profiles."""
    import sys, types
    try:
        import antenv.axon_hooks  # noqa: F401
        return
    except ImportError:
        pass
    try:
        from trn_agent_boot.trn_boot import _ntff_profile_via_ctypes
        hook = _ntff_profile_via_ctypes('/opt/axon/libaxon_pjrt.so')
    except Exception:
        hook = None
    import antenv
    mod = types.ModuleType("antenv.axon_hooks")
    mod.get_axon_ntff_profile_hook = lambda: hook
    mod.set_axon_ntff_profile_hook = lambda h: None
    antenv.axon_hooks = mod
    sys.modules["antenv.axon_hooks"] = mod


def _patch_tile_drain():
    """This image's walrus caps instructions at ONE sync wait.  Tile attaches
    one wait per outstanding semaphore to the exit drain and can give body
    instructions several waits.  Split: hoist all but one wait onto fresh
    single-wait NOPs inserted immediately before, on the same engine (engine
    streams are in-order, so this is semantics-preserving)."""
    import concourse.tile as tile
    import bass_rust
    from concourse.vector_clock import ScopedClock

    if getattr(tile.TileContext, "_drain_split_patched", False):
        return

    def patched(self, tick_clock, wait_clock):
        drain_inst = self.nc.sync.drain()
        wait_clock.add_sem_waits(
            drain_inst.ins, ScopedClock({None: tick_clock.global_clock})
        )
        si = drain_inst.ins.sync_info
        w = si.on_wait if si else []
        if len(w) > 1:
            drain_inst.ins.sync_info.on_wait = w[:1]
            for x in w[1:]:
                d2 = self.nc.sync.drain()
                d2.ins.sync_info = bass_rust.SyncInfo(on_wait=[x], on_update=[])
        self.nc.all_engine_barrier()
        assert self.sems is not None
        popped = self.nc._tile_sem_poison_stack.pop()
        assert popped is self._sem_poison
        self.nc.clear_and_free_semaphores(list(self.sems.allocated().values()))
        self.nc.all_engine_barrier()

    tile.TileContext._drain_and_barrier = patched

    from concourse import mybir
    from concourse.tile_scheduler import BassTileLoopBlock, BassTileRelease

    _special = [BassTileLoopBlock, BassTileRelease]
    for nm in ("BassTileCriticalSection", "BassTileBranchHintPlaceholder",
               "TileBranchInst", "BassTileConditionalBlock"):
        cls = getattr(tile, nm, None)
        if cls is not None:
            _special.append(cls)
    _special = tuple(_special)

    orig_lower = tile.TileContext._lower_ordered_insts

    def patched_lower(self, ordered):
        for bb_name in list(ordered.keys()):
            insts = ordered[bb_name]
            if not any(
                i.sync_info is not None and len(i.sync_info.on_wait) > 1
                for i in insts
            ):
                continue
            new = []
            for inst in insts:
                si = inst.sync_info
                if (
                    si is not None
                    and len(si.on_wait) > 1
                    and not isinstance(inst, _special)
                ):
                    waits = list(si.on_wait)
                    for x in waits[:-1]:
                        nop = mybir.InstNoOp(
                            name=self.nc.get_next_instruction_name(),
                            ins=[],
                            outs=[],
                            engine=inst.engine,
                            bass_nofuse=True,
                            sync_info=bass_rust.SyncInfo(on_wait=[x], on_update=[]),
                        )
                        new.append(nop)
                    si.on_wait = waits[-1:]
                new.append(inst)
            ordered[bb_name] = new
        return orig_lower(self, ordered)

    tile.TileContext._lower_ordered_insts = patched_lower
    tile.TileContext._drain_split_patched = True


def _build_program():
    """Build the per-core Bass program (same program for all 8 cores)."""
    global _PROG
    if _PROG is not None:
        return _PROG

    import concourse.bass as bass
    import concourse.tile as tile
    from concourse import mybir

    _patch_tile_drain()

    f32 = mybir.dt.float32
    f16 = mybir.dt.float16
    AFT = mybir.ActivationFunctionType

    nc = bass.Bass("TRN2", target_bir_lowering=False, debug=False)

    # x transposed per band and n-half: [band, h, w, r, n'] -> 32 KiB
    # contiguous per (b, h, w); a whole n-half loads as one 32 KiB/partition
    # descriptor per partition.
    xt_d = nc.dram_tensor("xt", [BPC, 2, W, PS, NSMP // 2], f16,
                          kind="ExternalInput").ap()
    # encode stationary, block-diag: [w, band, r, m]
    we_d = nc.dram_tensor("we", [W, BPC, PS, M], f16, kind="ExternalInput").ap()
    # decode stationary, block-diag: [m, band, f]
    wd_d = nc.dram_tensor("wd", [M, BPC, BANDW], f16, kind="ExternalInput").ap()
    # folded decode bias bd' = bd + Wd @ be, per-partition: [w', band, r']
    bdv_d = nc.dram_tensor("bdv", [W, BPC, PS], f32, kind="ExternalInput").ap()
    out_d = nc.dram_tensor("out", [BPC, PS, W, NSMP], f16, kind="ExternalOutput").ap()

    with tile.TileContext(nc) as tc:
        with (
            tc.tile_pool(name="singles", bufs=1) as singles,
            tc.tile_pool(name="zsb", bufs=4) as zpool,
            tc.tile_pool(name="outsb", bufs=12) as opool,
            tc.tile_pool(name="zps", bufs=1, space="PSUM") as zpsum,
            tc.tile_pool(name="ops", bufs=3, space="PSUM") as opsum,
        ):
            # SP issues DMAs in-order; sequence loads so each lands just
            # before its first consumer.  x(0,0) streams in r-quarters so
            # encode overlaps its own DMA.
            we_sb = singles.tile([W, BPC, PS, M], f16)
            nc.sync.dma_start(out=we_sb[:, 0], in_=we_d[:, 0])

            # preload the sigmoid ACT table off the critical path
            warm_act = singles.tile([1, 1], f16)
            nc.scalar.activation(out=warm_act, in_=we_sb[0:1, 0, 0, 0:1],
                                 func=AFT.Sigmoid)

            x_tiles = {}
            for bh in ((0, 0), (0, 1), (1, 0), (1, 1)):
                x_tiles[bh] = singles.tile(
                    [W, PS, NSMP // 2], f16, name=f"x{bh[0]}{bh[1]}")

            def load_x_quarter(b, h, q):
                # r-quarter of a half tile: 8 KiB runs; late ones issue
                # mid-decode so the FIFO DMA queues interleave input with
                # output stores instead of blocking them.
                nc.sync.dma_start(
                    out=x_tiles[(b, h)][:, 4 * q:4 * q + 4, :],
                    in_=xt_d[b, h, :, 4 * q:4 * q + 4, :])

            # All input issues up-front on the SP HWDGE ring; output stores
            # go out on the gpsimd SWDGE ring, so the DMA engines arbitrate
            # input/output fairly instead of FIFO-blocking stores behind
            # queued input.
            wd_sb = singles.tile([M, BPC, BANDW], f16)
            bdv_sb = singles.tile([W, BPC, PS], f32)
            for q in range(4):
                load_x_quarter(0, 0, q)
            nc.sync.dma_start(out=wd_sb[:, 0, :], in_=wd_d[:, 0, :])
            nc.sync.dma_start(out=bdv_sb, in_=bdv_d)
            nc.sync.dma_start(out=x_tiles[(0, 1)], in_=xt_d[0, 1])

            # PE warmup: start the clock ramp before x lands.  Dead stores
            # into an o_ps pool tile nothing ever reads.
            wp = opsum.tile([W, 1024], f32, name="o_ps")
            for i in range(14):
                nc.tensor.matmul(
                    wp[0:M, 512 * (i % 2):512 * (i % 2) + 512],
                    lhsT=we_sb[:, 0, i, :],
                    rhs=we_sb[:, 0, 8 * (i % 2):8 * (i % 2) + 8, :],
                    start=True, stop=True,
                )

            z_tiles = {}

            def enc_thunks(b, h, submajor=True):
                """One thunk per PE/DVE instruction of this half's encode.
                Matmul N is capped at 512, so each r feeds two matmuls into
                the two banks of a [64, 1024] z PSUM tile.  submajor orders
                all sub-0 matmuls first so its PSUM->fp16 cast overlaps the
                sub-1 matmuls (less serial latency at the unit transition);
                unit (0,0) keeps r-major order because its matmuls are gated
                by the streaming x r-quarters, which both subs consume."""
                X = x_tiles[(b, h)]
                z_ps = zpsum.tile([M, NSMP // 2], f32, name="z_ps")
                zsb_box = []

                def mk_mm(r, sub):
                    def mm():
                        nc.tensor.matmul(
                            z_ps[:, 512 * sub:512 * sub + 512],
                            lhsT=we_sb[:, b, r, :],
                            rhs=X[:, r, 512 * sub:512 * sub + 512],
                            start=(r == 0), stop=(r == PS - 1),
                        )
                    return mm

                def mk_cast(sub):
                    def cast():
                        if not zsb_box:
                            zsb_box.append(zpool.tile(
                                [M, NSMP // 2], f16, name="z_sb"))
                            z_tiles[(b, h)] = zsb_box[0]
                        nc.vector.tensor_copy(
                            zsb_box[0][:, 512 * sub:512 * sub + 512],
                            z_ps[:, 512 * sub:512 * sub + 512])
                    return cast

                th = []
                if submajor:
                    for sub in range(2):
                        th += [mk_mm(r, sub) for r in range(PS)]
                        th.append(mk_cast(sub))
                else:
                    for r in range(PS):
                        th += [mk_mm(r, 0), mk_mm(r, 1)]
                    th += [mk_cast(0), mk_cast(1)]
                return th

            def emit_dec(b, h, nxt, start_fc, sp_loads):
                """Decode+sigmoid for half (b, h); interleave the next
                half's encode thunks (PE) from start_fc on, and input
                dma_starts after specific output stores (the FIFO queues
                then serve input and output interleaved)."""
                for fc in range(PS):
                    o_ps = opsum.tile([W, 1024], f32, name="o_ps")
                    z_sb = z_tiles[(b, h)]
                    for sub in range(2):
                        nc.tensor.matmul(
                            o_ps[:, 512 * sub:512 * sub + 512],
                            lhsT=wd_sb[:, b, fc * W:(fc + 1) * W],
                            rhs=z_sb[:, 512 * sub:512 * sub + 512],
                            start=True, stop=True)
                    o_sb = opool.tile([W, NSMP // 2], f16, name="o_sb")
                    nc.scalar.activation(
                        out=o_sb, in_=o_ps, func=AFT.Sigmoid,
                        bias=bdv_sb[:, b, fc:fc + 1], scale=1.0,
                    )
                    nc.sync.dma_start(
                        out=out_d[b, fc, :, 1024 * h:1024 * h + 1024],
                        in_=o_sb)
                    for ld in sp_loads.get(fc, ()):
                        ld()
                    if nxt and fc >= start_fc:
                        slots_left = PS - fc
                        take = -(-len(nxt) // slots_left)   # ceil
                        for t in nxt[:take]:
                            t()
                        del nxt[:take]
                for t in nxt or []:
                    t()

            seq = [(0, 0), (0, 1), (1, 0), (1, 1)]
            start_fcs = [6, 4, 6]        # x-arrival-safe interleave points
            LQ = load_x_quarter
            SD = nc.sync.dma_start
            sp_plans = [
                # during dec(0,0): x(1,0) quarters spaced >= 3 stores apart
                # (one 1 MB quarter ~= 3 stores of FIFO time, so the o_sb
                # recycle never starves), band-1 weights between them
                {3: [lambda: LQ(1, 0, 0)],
                 6: [lambda: LQ(1, 0, 1)],
                 9: [lambda: LQ(1, 0, 2)],
                 10: [lambda: SD(out=we_sb[:, 1], in_=we_d[:, 1])],
                 12: [lambda: LQ(1, 0, 3)],
                 14: [lambda: SD(out=wd_sb[:, 1, :], in_=wd_d[:, 1, :])]},
                # during dec(0,1): x(1,1) quarters
                {3: [lambda: LQ(1, 1, 0)],
                 6: [lambda: LQ(1, 1, 1)],
                 9: [lambda: LQ(1, 1, 2)],
                 12: [lambda: LQ(1, 1, 3)]},
                {},
                {},
            ]
            for t in enc_thunks(0, 0, submajor=False):
                t()
            for i, (b, h) in enumerate(seq):
                nxt = enc_thunks(*seq[i + 1]) if i + 1 < len(seq) else None
                emit_dec(b, h, nxt, start_fcs[i] if nxt else PS, sp_plans[i])

    _PROG = nc
    return nc


def _host_prep(x, We, be, Wd, bd):
    """Slice/transpose/cast inputs into per-core maps (pure numpy)."""
    x = np.asarray(x, dtype=np.float32).reshape(NSMP, NPH, PS, W)
    We = np.asarray(We, dtype=np.float32)
    be = np.asarray(be, dtype=np.float32)
    Wd = np.asarray(Wd, dtype=np.float32)
    bd = np.asarray(bd, dtype=np.float32)

    # x -> [ph, h(n-half), w, r, n'] fp16
    x5 = x.reshape(2, NSMP // 2, NPH, PS, W)             # [h, n', ph, r, w]
    xt = x5.transpose(2, 0, 4, 3, 1)                     # [ph, h, w, r, n']

    # encode block-diag: wenc[ph, r, 16pw+c, 8pw+h] = We[ph*8+pw, h, r*16+c]
    We6 = We.reshape(NPH, NPW, HID, PS, PS)              # [ph, pw, h, r, c]
    wenc = np.zeros((NPH, PS, W, M), dtype=np.float32)
    for pw in range(NPW):
        wenc[:, :, PS * pw:PS * (pw + 1), HID * pw:HID * (pw + 1)] = (
            We6[:, pw].transpose(0, 2, 3, 1)             # [ph, r, c, h]
        )

    # decode block-diag: wdec[ph, 8pw+h, 128r'+16pw+c'] = Wd[ph*8+pw, r'*16+c', h]
    Wd5 = Wd.reshape(NPH, NPW, PS, PS, HID)              # [ph, pw, r', c', h]
    wdec = np.zeros((NPH, M, BANDW), dtype=np.float32)
    wdec_v = wdec.reshape(NPH, NPW, HID, PS, NPW, PS)
    for pw in range(NPW):
        wdec_v[:, pw, :, :, pw, :] = Wd5[:, pw].transpose(0, 3, 1, 2)  # [ph, h, r', c']

    # fold encode bias into decode bias: bd' = bd + Wd @ be  (exact, fp64)
    bdp = bd.astype(np.float64) + np.einsum(
        'ph,pdh->pd', be.astype(np.float64), Wd.astype(np.float64))
    bd4 = bdp.astype(np.float32).reshape(NPH, NPW, PS, PS)   # [ph, pw, r', c']
    bdv = bd4.transpose(1, 3, 0, 2).reshape(W, NPH, PS)      # [16pw+c', ph, r']

    in_maps = []
    for k in range(NCORES):
        sl = slice(BPC * k, BPC * (k + 1))
        in_maps.append({
            "xt": xt[sl].astype(np.float16, order='C'),
            "we": wenc[sl].transpose(2, 0, 1, 3).astype(np.float16, order='C'),
            "wd": wdec[sl].transpose(1, 0, 2).astype(np.float16, order='C'),
            "bdv": np.ascontiguousarray(bdv[:, sl, :]),
        })
    return in_maps


def kernel(x, We, be, Wd, bd, _trace=False):
    global LAST_EXEC_NS
    from concourse.bass_utils import run_bass_kernel_spmd

    if _trace:
        _install_ntff_hook()

    nc = _build_program()
    in_maps = _host_prep(x, We, be, Wd, bd)
    res = run_bass_kernel_spmd(nc, in_maps, list(range(NCORES)), trace=_trace)
    if _trace:
        LAST_EXEC_NS = res.exec_time_ns

    # out_k is out^T fp16: [band, r', w, n]  ->  out[n, band*2048 + 128 r' + w]
    out = np.empty((NSMP, H * W), dtype=np.float32)
    for k in range(NCORES):
        out[:, BPC * BANDW * k: BPC * BANDW * (k + 1)] = (
            res.results[k]["out"].reshape(BPC * BANDW, NSMP).T
        )
    return out.reshape(4, 512, H * W)


# revision 29
# speedup vs baseline: 1.0522x; 1.0522x over previous
"""Locally-connected autoencoder (128 independent 256->8->256 per-patch linears
+ sigmoid) on 8 Trainium2 NeuronCores.

Strategy
--------
Pure data parallel over the feature bands: core k owns image bands (2k, 2k+1)
for ALL 2048 samples.  The problem is HBM-bound (the 2.15 GFLOP of matmul is
trivial next to 512 MB of fp32 I/O), so the whole game is minimizing DMA
bytes:

* x ships as a SINGLE fp16 tensor (2 B/elem) instead of fp32 or a hi/lo bf16
  pair.  fp16's 10-bit mantissa keeps the max output rel-err ~4e-3, far
  inside the 2e-2 gate.  Host pre-transposes to [band, n-half, w, r, n'] so
  DMA descriptors move 8-32 KiB contiguous on both the HBM and SBUF side.
* the output is written back as fp16 (sigmoid output is in (0,1): always
  fp16-normal since |logit| <~ 6) and upcast on host - halves output bytes.
* the encode bias is folded into the decode bias on host
  (bd' = bd + Wd @ be), so the z path is a pure PSUM->SBUF fp16 cast (DVE)
  and ACT does nothing but Sigmoid+bias.

Per (band, n-half) unit - 2048 features x 1024 samples:
  encode:  z[64(pw,h), n1024] = sum_r Wenc_bd[r].T @ X[r]  (32 fp16 matmuls
           accumulated in PSUM; Wenc_bd is the block-diag arrangement of We)
  cast:    DVE copies z PSUM -> SBUF fp16 (no bias needed).
  decode:  out^T[128 f, n512] = Wdec_bd[:, fchunk].T @ z   (2 fp16 matmuls
           per fchunk; fchunk = one image row r')
  sigmoid: ACT reads decode PSUM [128, 1024], adds bd' as per-partition
           bias, writes fp16 out^T tiles -> DMA to HBM.

Scheduling (the whole kernel is a 4-stage software pipeline over units
(0,0),(0,1),(1,0),(1,1), paced by two facts measured from traces):
* DMA queues are FIFO per ring: input issued up-front blocks every output
  store behind it, stalling the o_sb recycle and ACT.  So only unit (0,0)'s
  x + band-0 weights + x(0,1) issue up-front; x(1,0)/x(1,1) r-quarters and
  band-1 weights issue from inside earlier decode loops, spaced >= 3 stores
  apart so each 1 MB quarter's FIFO block is absorbed by the o_sb depth.
* PE is in-order and ACT-backpressured during decode, so the next unit's
  encode matmuls are emitted interleaved between decode matmuls (start_fcs
  chosen so the thunks never wait on x, which would stall the stream), and
  finish just as ACT drains the current unit - ACT never idles between
  units.  x(0,0) streams as r-quarters so encode overlaps its own DMA.

Engine split: SP issues all DMAs (stores after their ACT sem, giving
just-in-time issue pacing), PE matmuls, DVE z-casts, ACT sigmoids only
(sigmoid table preloaded at t=0; identity/bias work eliminated by the
host-side bias fold).
"""

import numpy as np

# problem constants (hardcoded per contract)
H, W, PS = 256, 128, 16
NPH, NPW = H // PS, W // PS      # 16 bands, 8 patches/band
P, D, HID = NPH * NPW, PS * PS, 8
NSMP = 4 * 512                    # 2048 samples
BANDW = PS * W                    # 2048 floats per band per sample
NCORES = 8
BPC = NPH // NCORES               # 2 bands per core
M = NPW * HID                     # 64 latent rows per band

_PROG = None
LAST_EXEC_NS = None   # filled when kernel() runs with _trace=True


def _install_ntff_hook():
    """The agent image's antenv lacks axon_hooks; synthesize it so
    run_bass_kernel_spmd(trace=True) can capture NTFF profiles."""
    import sys, types
    try:
        import antenv.axon_hooks  # noqa: F401
        return
    except ImportError:
        pass
    try:
        from trn_agent_boot.trn_boot import _ntff_profile_via_ctypes
        hook = _ntff_profile_via_ctypes('/opt/axon/libaxon_pjrt.so')
    except Exception:
        hook = None
    import antenv
    mod = types.ModuleType("antenv.axon_hooks")
    mod.get_axon_ntff_profile_hook = lambda: hook
    mod.set_axon_ntff_profile_hook = lambda h: None
    antenv.axon_hooks = mod
    sys.modules["antenv.axon_hooks"] = mod


def _patch_tile_drain():
    """This image's walrus caps instructions at ONE sync wait.  Tile attaches
    one wait per outstanding semaphore to the exit drain and can give body
    instructions several waits.  Split: hoist all but one wait onto fresh
    single-wait NOPs inserted immediately before, on the same engine (engine
    streams are in-order, so this is semantics-preserving)."""
    import concourse.tile as tile
    import bass_rust
    from concourse.vector_clock import ScopedClock

    if getattr(tile.TileContext, "_drain_split_patched", False):
        return

    def patched(self, tick_clock, wait_clock):
        drain_inst = self.nc.sync.drain()
        wait_clock.add_sem_waits(
            drain_inst.ins, ScopedClock({None: tick_clock.global_clock})
        )
        si = drain_inst.ins.sync_info
        w = si.on_wait if si else []
        if len(w) > 1:
            drain_inst.ins.sync_info.on_wait = w[:1]
            for x in w[1:]:
                d2 = self.nc.sync.drain()
                d2.ins.sync_info = bass_rust.SyncInfo(on_wait=[x], on_update=[])
        self.nc.all_engine_barrier()
        assert self.sems is not None
        popped = self.nc._tile_sem_poison_stack.pop()
        assert popped is self._sem_poison
        self.nc.clear_and_free_semaphores(list(self.sems.allocated().values()))
        self.nc.all_engine_barrier()

    tile.TileContext._drain_and_barrier = patched

    from concourse import mybir
    from concourse.tile_scheduler import BassTileLoopBlock, BassTileRelease

    _special = [BassTileLoopBlock, BassTileRelease]
    for nm in ("BassTileCriticalSection", "BassTileBranchHintPlaceholder",
               "TileBranchInst", "BassTileConditionalBlock"):
        cls = getattr(tile, nm, None)
        if cls is not None:
            _special.append(cls)
    _special = tuple(_special)

    orig_lower = tile.TileContext._lower_ordered_insts

    def patched_lower(self, ordered):
        for bb_name in list(ordered.keys()):
            insts = ordered[bb_name]
            if not any(
                i.sync_info is not None and len(i.sync_info.on_wait) > 1
                for i in insts
            ):
                continue
            new = []
            for inst in insts:
                si = inst.sync_info
                if (
                    si is not None
                    and len(si.on_wait) > 1
                    and not isinstance(inst, _special)
                ):
                    waits = list(si.on_wait)
                    for x in waits[:-1]:
                        nop = mybir.InstNoOp(
                            name=self.nc.get_next_instruction_name(),
                            ins=[],
                            outs=[],
                            engine=inst.engine,
                            bass_nofuse=True,
                            sync_info=bass_rust.SyncInfo(on_wait=[x], on_update=[]),
                        )
                        new.append(nop)
                    si.on_wait = waits[-1:]
                new.append(inst)
            ordered[bb_name] = new
        return orig_lower(self, ordered)

    tile.TileContext._lower_ordered_insts = patched_lower
    tile.TileContext._drain_split_patched = True


def _build_program():
    """Build the per-core Bass program (same program for all 8 cores)."""
    global _PROG
    if _PROG is not None:
        return _PROG

    import concourse.bass as bass
    import concourse.tile as tile
    from concourse import mybir

    _patch_tile_drain()

    f32 = mybir.dt.float32
    f16 = mybir.dt.float16
    AFT = mybir.ActivationFunctionType

    nc = bass.Bass("TRN2", target_bir_lowering=False, debug=False)

    # x transposed per band and n-half: [band, h, w, r, n'] -> 32 KiB
    # contiguous per (b, h, w); a whole n-half loads as one 32 KiB/partition
    # descriptor per partition.
    xt_d = nc.dram_tensor("xt", [BPC, 2, W, PS, NSMP // 2], f16,
                          kind="ExternalInput").ap()
    # encode stationary, block-diag: [w, band, r, m]
    we_d = nc.dram_tensor("we", [W, BPC, PS, M], f16, kind="ExternalInput").ap()
    # decode stationary, block-diag: [m, band, f]
    wd_d = nc.dram_tensor("wd", [M, BPC, BANDW], f16, kind="ExternalInput").ap()
    # folded decode bias bd' = bd + Wd @ be, per-partition: [w', band, r']
    bdv_d = nc.dram_tensor("bdv", [W, BPC, PS], f32, kind="ExternalInput").ap()
    out_d = nc.dram_tensor("out", [BPC, PS, W, NSMP], f16, kind="ExternalOutput").ap()

    with tile.TileContext(nc) as tc:
        with (
            tc.tile_pool(name="singles", bufs=1) as singles,
            tc.tile_pool(name="zsb", bufs=4) as zpool,
            tc.tile_pool(name="outsb", bufs=12) as opool,
            tc.tile_pool(name="zps", bufs=1, space="PSUM") as zpsum,
            tc.tile_pool(name="ops", bufs=3, space="PSUM") as opsum,
        ):
            # SP issues DMAs in-order; sequence loads so each lands just
            # before its first consumer.  x(0,0) streams in r-quarters so
            # encode overlaps its own DMA.
            we_sb = singles.tile([W, BPC, PS, M], f16)
            nc.sync.dma_start(out=we_sb[:, 0], in_=we_d[:, 0])

            # preload the sigmoid ACT table off the critical path
            warm_act = singles.tile([1, 1], f16)
            nc.scalar.activation(out=warm_act, in_=we_sb[0:1, 0, 0, 0:1],
                                 func=AFT.Sigmoid)

            x_tiles = {}
            for bh in ((0, 0), (0, 1), (1, 0), (1, 1)):
                x_tiles[bh] = singles.tile(
                    [W, PS, NSMP // 2], f16, name=f"x{bh[0]}{bh[1]}")

            def load_x_quarter(b, h, q):
                # r-quarter of a half tile: 8 KiB runs; late ones issue
                # mid-decode so the FIFO DMA queues interleave input with
                # output stores instead of blocking them.
                nc.sync.dma_start(
                    out=x_tiles[(b, h)][:, 4 * q:4 * q + 4, :],
                    in_=xt_d[b, h, :, 4 * q:4 * q + 4, :])

            # All input issues up-front on the SP HWDGE ring; output stores
            # go out on the gpsimd SWDGE ring, so the DMA engines arbitrate
            # input/output fairly instead of FIFO-blocking stores behind
            # queued input.
            wd_sb = singles.tile([M, BPC, BANDW], f16)
            bdv_sb = singles.tile([W, BPC, PS], f32)
            for q in range(4):
                load_x_quarter(0, 0, q)
            nc.sync.dma_start(out=wd_sb[:, 0, :], in_=wd_d[:, 0, :])
            nc.sync.dma_start(out=bdv_sb, in_=bdv_d)
            nc.sync.dma_start(out=x_tiles[(0, 1)], in_=xt_d[0, 1])

            # PE warmup: start the clock ramp before x lands.  Dead stores
            # into an o_ps pool tile nothing ever reads.
            wp = opsum.tile([W, 1024], f32, name="o_ps")
            for i in range(14):
                nc.tensor.matmul(
                    wp[0:M, 512 * (i % 2):512 * (i % 2) + 512],
                    lhsT=we_sb[:, 0, i, :],
                    rhs=we_sb[:, 0, 8 * (i % 2):8 * (i % 2) + 8, :],
                    start=True, stop=True,
                )

            z_tiles = {}

            def enc_thunks(b, h):
                """One thunk per PE/DVE instruction of this half's encode.
                Matmul N is capped at 512, so each r feeds two matmuls into
                the two banks of a [64, 1024] z PSUM tile; the two halves
                cast to fp16 separately so the first cast overlaps the
                tail matmuls."""
                X = x_tiles[(b, h)]
                z_ps = zpsum.tile([M, NSMP // 2], f32, name="z_ps")
                zsb_box = []

                def mk_mm(r, sub):
                    def mm():
                        nc.tensor.matmul(
                            z_ps[:, 512 * sub:512 * sub + 512],
                            lhsT=we_sb[:, b, r, :],
                            rhs=X[:, r, 512 * sub:512 * sub + 512],
                            start=(r == 0), stop=(r == PS - 1),
                        )
                    return mm

                def mk_cast(sub):
                    def cast():
                        if not zsb_box:
                            zsb_box.append(zpool.tile(
                                [M, NSMP // 2], f16, name="z_sb"))
                            z_tiles[(b, h)] = zsb_box[0]
                        nc.vector.tensor_copy(
                            zsb_box[0][:, 512 * sub:512 * sub + 512],
                            z_ps[:, 512 * sub:512 * sub + 512])
                    return cast

                th = []
                for r in range(PS):
                    th += [mk_mm(r, 0), mk_mm(r, 1)]
                th += [mk_cast(0), mk_cast(1)]
                return th

            def emit_dec(b, h, nxt, start_fc, sp_loads):
                """Decode+sigmoid for half (b, h); interleave the next
                half's encode thunks (PE) from start_fc on, and input
                dma_starts after specific output stores (the FIFO queues
                then serve input and output interleaved)."""
                for fc in range(PS):
                    o_ps = opsum.tile([W, 1024], f32, name="o_ps")
                    z_sb = z_tiles[(b, h)]
                    for sub in range(2):
                        nc.tensor.matmul(
                            o_ps[:, 512 * sub:512 * sub + 512],
                            lhsT=wd_sb[:, b, fc * W:(fc + 1) * W],
                            rhs=z_sb[:, 512 * sub:512 * sub + 512],
                            start=True, stop=True)
                    o_sb = opool.tile([W, NSMP // 2], f16, name="o_sb")
                    nc.scalar.activation(
                        out=o_sb, in_=o_ps, func=AFT.Sigmoid,
                        bias=bdv_sb[:, b, fc:fc + 1], scale=1.0,
                    )
                    nc.sync.dma_start(
                        out=out_d[b, fc, :, 1024 * h:1024 * h + 1024],
                        in_=o_sb)
                    for ld in sp_loads.get(fc, ()):
                        ld()
                    if nxt and fc >= start_fc:
                        slots_left = PS - fc
                        take = -(-len(nxt) // slots_left)   # ceil
                        for t in nxt[:take]:
                            t()
                        del nxt[:take]
                for t in nxt or []:
                    t()

            seq = [(0, 0), (0, 1), (1, 0), (1, 1)]
            start_fcs = [6, 4, 6]        # x-arrival-safe interleave points
            LQ = load_x_quarter
            SD = nc.sync.dma_start
            sp_plans = [
                # during dec(0,0): x(1,0) quarters spaced >= 3 stores apart
                # (one 1 MB quarter ~= 3 stores of FIFO time, so the o_sb
                # recycle never starves), band-1 weights between them
                {3: [lambda: LQ(1, 0, 0)],
                 6: [lambda: LQ(1, 0, 1)],
                 9: [lambda: LQ(1, 0, 2)],
                 10: [lambda: SD(out=we_sb[:, 1], in_=we_d[:, 1])],
                 12: [lambda: LQ(1, 0, 3)],
                 14: [lambda: SD(out=wd_sb[:, 1, :], in_=wd_d[:, 1, :])]},
                # during dec(0,1): x(1,1) quarters
                {3: [lambda: LQ(1, 1, 0)],
                 6: [lambda: LQ(1, 1, 1)],
                 9: [lambda: LQ(1, 1, 2)],
                 12: [lambda: LQ(1, 1, 3)]},
                {},
                {},
            ]
            for t in enc_thunks(0, 0):
                t()
            for i, (b, h) in enumerate(seq):
                nxt = enc_thunks(*seq[i + 1]) if i + 1 < len(seq) else None
                emit_dec(b, h, nxt, start_fcs[i] if nxt else PS, sp_plans[i])

    _PROG = nc
    return nc


def _host_prep(x, We, be, Wd, bd):
    """Slice/transpose/cast inputs into per-core maps (pure numpy)."""
    x = np.asarray(x, dtype=np.float32).reshape(NSMP, NPH, PS, W)
    We = np.asarray(We, dtype=np.float32)
    be = np.asarray(be, dtype=np.float32)
    Wd = np.asarray(Wd, dtype=np.float32)
    bd = np.asarray(bd, dtype=np.float32)

    # x -> [ph, h(n-half), w, r, n'] fp16
    x5 = x.reshape(2, NSMP // 2, NPH, PS, W)             # [h, n', ph, r, w]
    xt = x5.transpose(2, 0, 4, 3, 1)                     # [ph, h, w, r, n']

    # encode block-diag: wenc[ph, r, 16pw+c, 8pw+h] = We[ph*8+pw, h, r*16+c]
    We6 = We.reshape(NPH, NPW, HID, PS, PS)              # [ph, pw, h, r, c]
    wenc = np.zeros((NPH, PS, W, M), dtype=np.float32)
    for pw in range(NPW):
        wenc[:, :, PS * pw:PS * (pw + 1), HID * pw:HID * (pw + 1)] = (
            We6[:, pw].transpose(0, 2, 3, 1)             # [ph, r, c, h]
        )

    # decode block-diag: wdec[ph, 8pw+h, 128r'+16pw+c'] = Wd[ph*8+pw, r'*16+c', h]
    Wd5 = Wd.reshape(NPH, NPW, PS, PS, HID)              # [ph, pw, r', c', h]
    wdec = np.zeros((NPH, M, BANDW), dtype=np.float32)
    wdec_v = wdec.reshape(NPH, NPW, HID, PS, NPW, PS)
    for pw in range(NPW):
        wdec_v[:, pw, :, :, pw, :] = Wd5[:, pw].transpose(0, 3, 1, 2)  # [ph, h, r', c']

    # fold encode bias into decode bias: bd' = bd + Wd @ be  (exact, fp64)
    bdp = bd.astype(np.float64) + np.einsum(
        'ph,pdh->pd', be.astype(np.float64), Wd.astype(np.float64))
    bd4 = bdp.astype(np.float32).reshape(NPH, NPW, PS, PS)   # [ph, pw, r', c']
    bdv = bd4.transpose(1, 3, 0, 2).reshape(W, NPH, PS)      # [16pw+c', ph, r']

    in_maps = []
    for k in range(NCORES):
        sl = slice(BPC * k, BPC * (k + 1))
        in_maps.append({
            "xt": xt[sl].astype(np.float16, order='C'),
            "we": wenc[sl].transpose(2, 0, 1, 3).astype(np.float16, order='C'),
            "wd": wdec[sl].transpose(1, 0, 2).astype(np.float16, order='C'),
            "bdv": np.ascontiguousarray(bdv[:, sl, :]),
        })
    return in_maps


def kernel(x, We, be, Wd, bd, _trace=False):
    global LAST_EXEC_NS
    from concourse.bass_utils import run_bass_kernel_spmd

    if _trace:
        _install_ntff_hook()

    nc = _build_program()
    in_maps = _host_prep(x, We, be, Wd, bd)
    res = run_bass_kernel_spmd(nc, in_maps, list(range(NCORES)), trace=_trace)
    if _trace:
        LAST_EXEC_NS = res.exec_time_ns

    # out_k is out^T fp16: [band, r', w, n]  ->  out[n, band*2048 + 128 r' + w]
    out = np.empty((NSMP, H * W), dtype=np.float32)
    for k in range(NCORES):
        out[:, BPC * BANDW * k: BPC * BANDW * (k + 1)] = (
            res.results[k]["out"].reshape(BPC * BANDW, NSMP).T
        )
    return out.reshape(4, 512, H * W)
